# revision 1
# baseline (speedup 1.0000x reference)
"""Block-circulant linear layer (CirculantLinear) Trainium2 kernel.

y = x @ W^T + bias where W is built from a 256x256 grid of 8x8 circulant
blocks given by per-block eigenvalue vectors `eigens` [256, 256, 8].

Math: per-block circulant multiply diagonalizes under the length-8 rFFT:
  Yf[b, i, bin] = sum_j Xf[b, j, bin] * Ef[i, j, bin]
which is, per frequency bin, a [B,256] x [256,256] (complex) matmul —
~4.5x fewer FLOPs than materializing the dense 2048x2048 W.

Device pipeline (per core, data-parallel over batch, 8 cores):
  T-in : PE transposes x (batch-major -> channel-major), bf16
  S2   : block-diag rFFT8 matmul (one shared 128x128 stationary)
  P1   : SBUF->SBUF DMA partition regroup (interleaved -> bin-pair grouped)
  S3   : 64 dense 128x128xBC matmuls in frequency domain (the core work)
  P2   : regroup back (bin-pair -> interleaved)
  S4   : fused iFFT8 + transpose-out (activation-stationary matmuls),
         producing batch-major y in fp32

Layout (32-partition move units, re/im of each bin paired so every
SBUF slice starts at a 0/32/64/96 partition boundary):
  xT group g:    p = j16*8 + k            (channels g*128..g*128+127)
  Xf group g:    p = P*32 + j16*2 + c     (freq comp fc = 2P + c)
  Xb tile (P,jq): p = gg*32 + j16*2 + c   (j = jq*64 + gg*16 + j16)
  Yb tile (P,iq): p = uu*32 + i16*2 + c'  (i = iq*64 + uu*16 + i16)
  Yi group h:    p = P*32 + i16*2 + c'    (i = h*16 + i16)
  y[:, h*128+i16*8+t] comes from Yi[h].T @ BDi
"""

import hashlib
import os
import shutil
from contextlib import ExitStack

import ml_dtypes
import numpy as np

import bass_rust
import concourse.bass as bass
import concourse.mybir as mybir
import concourse.tile as tile
from concourse.vector_clock import ScopedClock

BF16 = ml_dtypes.bfloat16

N_CORES = 8
B_FULL, C = 16384, 2048
BPC = B_FULL // N_CORES  # rows per core
BC = 512  # batch chunk
SUB = BC // 128  # 128-row subtiles per chunk


# ---------------------------------------------------------------------------
# Environment patches (applied once on import)
# ---------------------------------------------------------------------------

def _patched_drain_and_barrier(self, tick_clock, wait_clock):
    # The stock version attaches every outstanding sem wait to one SP Drain;
    # this walrus build rejects >1 sync wait on a CTRL instruction, so spread
    # the waits across a chain of drains.
    nc = self.nc
    drain_inst = nc.sync.drain()
    wait_clock.add_sem_waits(
        drain_inst.ins, ScopedClock({None: tick_clock.global_clock})
    )
    si = drain_inst.ins.sync_info
    waits = list(si.on_wait) if si and si.on_wait else []
    if len(waits) > 1:
        si.on_wait = waits[:1]
        for i in range(1, len(waits)):
            extra = nc.sync.drain()
            extra.ins.sync_info = bass_rust.SyncInfo(
                on_wait=waits[i : i + 1], on_update=[]
            )
    nc.all_engine_barrier()
    assert self.sems is not None
    popped = nc._tile_sem_poison_stack.pop()
    assert popped is self._sem_poison
    nc.clear_and_free_semaphores(list(self.sems.allocated().values()))
    nc.all_engine_barrier()


tile.TileContext._drain_and_barrier = _patched_drain_and_barrier

_MAX_WAITS = 1  # this walrus build rejects >1 sync wait per instruction


def _split_sync_waits(nc, maxw=_MAX_WAITS):
    """Walrus here supports few sync waits per instruction; hoist the excess
    onto same-engine NoOps inserted immediately before the instruction."""
    ctr = 0
    for f in nc.m.functions:
        for bb in f.blocks:
            il = bb.instructions
            out = []
            changed = False
            for inst in il:
                si = inst.sync_info
                waits = list(si.on_wait) if si and si.on_wait else []
                if len(waits) > maxw:
                    si.on_wait = waits[:maxw]
                    for i in range(maxw, len(waits), maxw):
                        ctr += 1
                        nop = mybir.InstNoOp(name=f"waitnop-{ctr}", ins=[], outs=[])
                        nop.engine = inst.engine
                        nop.sync_info = bass_rust.SyncInfo(
                            on_wait=waits[i : i + maxw], on_update=[]
                        )
                        out.append(nop)
                    changed = True
                out.append(inst)
            if changed:
                bb.instructions = out


def _install_neff_cache():
    # Persistent on-disk NEFF cache keyed on BIR content: saves the ~3-10 min
    # walrus compile across processes when the kernel is unchanged.
    import concourse.bass2jax as b2j
    from concourse import bass_utils as bu

    orig = bu.compile_bir_kernel
    cache_dir = os.environ.get(
        "BASS_NEFF_CACHE", os.path.join(os.path.expanduser("~"), ".cache", "bass_neff")
    )

    def cached(bir_json, tmpdir, neff_name="file.neff"):
        try:
            os.makedirs(cache_dir, exist_ok=True)
            h = hashlib.sha256(bir_json).hexdigest()[:32]
            src = os.path.join(cache_dir, h + ".neff")
            if os.path.exists(src):
                dst = os.path.join(tmpdir, neff_name)
                shutil.copy(src, dst)
                return dst
            p = orig(bir_json, tmpdir, neff_name)
            shutil.copy(p, src)
            return p
        except OSError:
            return orig(bir_json, tmpdir, neff_name)

    b2j.compile_bir_kernel = cached
    bu.compile_bir_kernel = cached


_install_neff_cache()


# ---------------------------------------------------------------------------
# Host-side constant construction
# ---------------------------------------------------------------------------

def _make_F8():
    # packed rfft rows: fc=0: X0; fc=1: X4; fc=2m/2m+1: bin m re/im
    F = np.zeros((8, 8), np.float64)
    k = np.arange(8)
    F[0] = 1.0
    F[1] = (-1.0) ** k
    for m in (1, 2, 3):
        F[2 * m] = np.cos(2 * np.pi * m * k / 8)
        F[2 * m + 1] = -np.sin(2 * np.pi * m * k / 8)
    return F


def _make_F8inv():
    Fi = np.zeros((8, 8), np.float64)  # [t, fc]
    t = np.arange(8)
    Fi[:, 0] = 1 / 8
    Fi[:, 1] = ((-1.0) ** t) / 8
    for m in (1, 2, 3):
        Fi[:, 2 * m] = (2 / 8) * np.cos(2 * np.pi * m * t / 8)
        Fi[:, 2 * m + 1] = -(2 / 8) * np.sin(2 * np.pi * m * t / 8)
    return Fi


def make_bd():
    """S2 stationary lhsT [128 (j16,k), 128 (P,j16,c)]."""
    F = _make_F8()
    BD = np.zeros((128, 128), np.float64)
    for j16 in range(16):
        for k in range(8):
            for P in range(4):
                for c in range(2):
                    BD[j16 * 8 + k, P * 32 + j16 * 2 + c] = F[2 * P + c, k]
    return BD.astype(BF16)


def make_bdi():
    """S4 moving rhs [128 (P,i16,c'), 128 (i16,t)]."""
    Fi = _make_F8inv()
    BDi = np.zeros((128, 128), np.float64)
    for i16 in range(16):
        for P in range(4):
            for c in range(2):
                for t in range(8):
                    BDi[P * 32 + i16 * 2 + c, i16 * 8 + t] = Fi[t, 2 * P + c]
    return BDi.astype(BF16)


def make_w(eigens):
    """S3 weights, packed [128, 64*128]: block (P,iq,jq) at cols
    (P*16+iq*4+jq)*128; W[pj, pi] = M[c(pj)][c'(pi)][i(pi), j(pj)]."""
    Ef = np.fft.rfft(np.asarray(eigens, np.float64), axis=-1)  # [gy, gx, 5]
    E0 = Ef[:, :, 0].real
    E4 = Ef[:, :, 4].real
    Z = np.zeros_like(E0)

    pl = np.arange(128)
    quad, r = pl // 32, pl % 32
    s16, comp = r // 2, r % 2

    w = np.zeros((128, 64 * 128), np.float64)
    for P in range(4):
        if P == 0:
            M = np.array([[E0, Z], [Z, E4]])  # [c, c', i, j]
        else:
            Er, Ei = Ef[:, :, P].real, Ef[:, :, P].imag
            M = np.array([[Er, Ei], [-Ei, Er]])
        for iq in range(4):
            ii = iq * 64 + quad * 16 + s16  # per-col global i
            for jq in range(4):
                jj = jq * 64 + quad * 16 + s16  # per-row global j
                blk = M[comp[:, None], comp[None, :], ii[None, :], jj[:, None]]
                base = (P * 16 + iq * 4 + jq) * 128
                w[:, base : base + 128] = blk
    return w.astype(BF16)


# ---------------------------------------------------------------------------
# Device kernel
# ---------------------------------------------------------------------------

def build_nc(rows=BPC, repeat=1, split_waits=True, loop_T=1, cfg=None):
    cfg = dict(cfg or {})
    load_eng = cfg.get("load_eng", "gpsimd")
    store_eng = cfg.get("store_eng", "scalar")
    p1_eng = cfg.get("p1_eng", "gpsimd")
    p2_eng = cfg.get("p2_eng", "sync")
    tp_bufs = cfg.get("tp_bufs", 2)
    s3_bufs = cfg.get("s3_bufs", 2)
    s4_bufs = cfg.get("s4_bufs", 2)
    xbm_bufs = cfg.get("xbm_bufs", 2)
    xf_bufs = cfg.get("xf_bufs", 2)
    xb_bufs = cfg.get("xb_bufs", 2)
    yb_bufs = cfg.get("yb_bufs", 2)
    yi_bufs = cfg.get("yi_bufs", 2)
    ysb_bufs = cfg.get("ysb_bufs", 3)
    from contextlib import nullcontext

    f32 = mybir.dt.float32
    bf16 = mybir.dt.bfloat16
    nchunk = rows // BC

    nc = bass.Bass("TRN2", target_bir_lowering=False, debug=False, num_devices=N_CORES)
    x_d = nc.declare_dram_parameter("x", [rows, C], bf16, isOutput=False)
    id_d = nc.declare_dram_parameter("ident", [128, 128], bf16, isOutput=False)
    bd_d = nc.declare_dram_parameter("bd", [128, 128], bf16, isOutput=False)
    bdi_d = nc.declare_dram_parameter("bdi", [128, 128], bf16, isOutput=False)
    w_d = nc.declare_dram_parameter("w", [128, 64 * 128], bf16, isOutput=False)
    y_d = nc.declare_dram_parameter("y", [rows, C], bf16, isOutput=True)

    with tile.TileContext(nc) as tc, ExitStack() as ctx:
        cpool = ctx.enter_context(tc.tile_pool(name="consts", bufs=1))
        ident = cpool.tile([128, 128], bf16)
        nc.sync.dma_start(ident[:], id_d.ap())
        bd = cpool.tile([128, 128], bf16)
        nc.sync.dma_start(bd[:], bd_d.ap())
        bdi = cpool.tile([128, 128], bf16)
        nc.sync.dma_start(bdi[:], bdi_d.ap())
        w = cpool.tile([128, 64 * 128], bf16)
        nc.sync.dma_start(w[:], w_d.ap())

        xbm_pool = ctx.enter_context(tc.tile_pool(name="xbm", bufs=xbm_bufs))
        tp_ps = ctx.enter_context(tc.tile_pool(name="tp_ps", bufs=tp_bufs, space="PSUM"))
        xT_pool = ctx.enter_context(tc.tile_pool(name="xT", bufs=1))
        s2_ps = ctx.enter_context(tc.tile_pool(name="s2_ps", bufs=2, space="PSUM"))
        xf_pool = ctx.enter_context(tc.tile_pool(name="xf", bufs=xf_bufs))
        xb_pool = ctx.enter_context(tc.tile_pool(name="xb", bufs=xb_bufs))
        s3_ps = ctx.enter_context(tc.tile_pool(name="s3_ps", bufs=s3_bufs, space="PSUM"))
        yb_pool = ctx.enter_context(tc.tile_pool(name="yb", bufs=yb_bufs))
        yi_pool = ctx.enter_context(tc.tile_pool(name="yi", bufs=yi_bufs))
        s4_ps = ctx.enter_context(tc.tile_pool(name="s4_ps", bufs=s4_bufs, space="PSUM"))
        y_pool = ctx.enter_context(tc.tile_pool(name="ysb", bufs=ysb_bufs))

        def body():
            for ch in range(repeat * nchunk):
                row0 = (ch % nchunk) * BC

                # load chunk batch-major (x pre-cast to bf16 on host)
                xbm = xbm_pool.tile([128, SUB * C], bf16)
                for s in range(SUB):
                    getattr(nc, load_eng).dma_start(
                        xbm[:, s * C : (s + 1) * C],
                        x_d.ap()[row0 + s * 128 : row0 + (s + 1) * 128, :],
                    )

                # T-in: PE transpose to channel-major
                xT = xT_pool.tile([128, 16 * BC], bf16)
                for g in range(16):
                    pt = tp_ps.tile([128, SUB * 128], bf16)
                    for s in range(SUB):
                        nc.tensor.matmul(
                            pt[:, s * 128 : (s + 1) * 128],
                            xbm[:, s * C + g * 128 : s * C + (g + 1) * 128],
                            ident[:],
                            is_transpose=True,
                            start=(s == 0),
                            stop=(s == SUB - 1),
                        )
                    nc.vector.tensor_copy(xT[:, g * BC : (g + 1) * BC], pt[:])

                # S2: rFFT8 along each 8-channel block (block-diag stationary)
                xf = xf_pool.tile([128, 16 * BC], bf16)
                for g in range(16):
                    ps = s2_ps.tile([128, BC], f32)
                    nc.tensor.matmul(
                        ps[:], bd[:], xT[:, g * BC : (g + 1) * BC],
                        start=True, stop=True,
                    )
                    nc.scalar.copy(xf[:, g * BC : (g + 1) * BC], ps[:])

                # P1: regroup interleaved -> bin-pair tiles.
                # One DMA per (P, gg) moves the 4 groups g = jq*4+gg:
                #   src xf[P*32:+32, (g b)] strided over g (step 4*BC)
                #   dst xb[gg*32:+32, (P*4+jq)*BC] consecutive over jq (step BC)
                xb = xb_pool.tile([128, 16 * BC], bf16)
                xf3 = xf[:].rearrange("p (g b) -> p g b", g=16)
                xb3 = xb[:].rearrange("p (t b) -> p t b", t=16)
                for P in range(4):
                    for gg in range(4):
                        getattr(nc, p1_eng).dma_start(
                            xb3[gg * 32 : (gg + 1) * 32, P * 4 : P * 4 + 4, :],
                            xf3[P * 32 : (P + 1) * 32, gg :: 4, :],
                        )

                # S3: frequency-domain block matmuls
                yb = yb_pool.tile([128, 16 * BC], bf16)
                for P in range(4):
                    for iq in range(4):
                        ps = s3_ps.tile([128, BC], f32)
                        for jq in range(4):
                            base = (P * 16 + iq * 4 + jq) * 128
                            nc.tensor.matmul(
                                ps[:],
                                w[:, base : base + 128],
                                xb[:, (P * 4 + jq) * BC : (P * 4 + jq + 1) * BC],
                                start=(jq == 0),
                                stop=(jq == 3),
                            )
                        nc.vector.tensor_copy(
                            yb[:, (P * 4 + iq) * BC : (P * 4 + iq + 1) * BC], ps[:]
                        )

                # P2: regroup bin-pair -> interleaved output groups.
                # One DMA per (P, uu) moves the 4 tiles iq = 0..3 into the 4
                # groups h = iq*4 + uu (dst strided over h, step 4 tiles).
                yi = yi_pool.tile([128, 16 * BC], bf16)
                yb3 = yb[:].rearrange("p (t b) -> p t b", t=16)
                yi3 = yi[:].rearrange("p (h b) -> p h b", h=16)
                for P in range(4):
                    for uu in range(4):
                        getattr(nc, p2_eng).dma_start(
                            yi3[P * 32 : (P + 1) * 32, uu :: 4, :],
                            yb3[uu * 32 : (uu + 1) * 32, P * 4 : P * 4 + 4, :],
                        )

                # S4: fused iFFT8 + transpose back to batch-major (fp32 out)
                for s in range(SUB):
                    ysb = y_pool.tile([128, C], bf16)
                    for hq in range(4):
                        ps = s4_ps.tile([128, 512], f32)
                        for u in range(4):
                            h = hq * 4 + u
                            nc.tensor.matmul(
                                ps[:, u * 128 : (u + 1) * 128],
                                yi[:, h * BC + s * 128 : h * BC + (s + 1) * 128],
                                bdi[:],
                                start=(u == 0),
                                stop=(u == 3),
                            )
                        nc.scalar.copy(ysb[:, hq * 512 : (hq + 1) * 512], ps[:])
                    getattr(nc, store_eng).dma_start(
                        y_d.ap()[row0 + s * 128 : row0 + (s + 1) * 128, :], ysb[:]
                    )

        if loop_T > 1:
            with tc.For_i(0, loop_T, 1):
                body()
        else:
            body()

    if split_waits:
        _split_sync_waits(nc)
    return nc


# ---------------------------------------------------------------------------
# Host wrapper
# ---------------------------------------------------------------------------


_NC_CACHE = {}


def _get_nc(rows=BPC):
    if rows not in _NC_CACHE:
        _NC_CACHE[rows] = build_nc(rows)
    return _NC_CACHE[rows]


_CONSTS = None


def _static_consts():
    global _CONSTS
    if _CONSTS is None:
        _CONSTS = {
            "ident": np.eye(128, dtype=BF16),
            "bd": make_bd(),
            "bdi": make_bdi(),
        }
    return _CONSTS


def kernel(x, eigens, bias):
    from concourse.bass_utils import run_bass_kernel_spmd

    x = np.asarray(x, np.float32).astype(BF16)  # device consumes bf16
    bias = np.asarray(bias, np.float32)
    consts = dict(_static_consts())
    consts["w"] = make_w(eigens)

    nc = _get_nc(BPC)
    in_maps = [
        {"x": np.ascontiguousarray(x[i * BPC : (i + 1) * BPC]), **consts}
        for i in range(N_CORES)
    ]
    res = run_bass_kernel_spmd(nc, in_maps, list(range(N_CORES)))
    y = np.concatenate([r["y"] for r in res.results], axis=0).astype(np.float32)
    if np.any(bias):
        y = y + bias
    return y.astype(np.float32, copy=False)

